# revision 1
# baseline (speedup 1.0000x reference)
"""Multi-head attention (N=2, S=2048, D=1024, H=16) on 8 TRN2 NeuronCores.

Sharding: core c handles batch b = c//4 and head group g = c%4 (4 heads).
Column-parallel qkv projection, per-head attention, row-parallel o_proj;
the 4 partial o_proj outputs per batch are summed on the host (unshard).

Per-core device kernel (bf16 matmul operands, fp32 PSUM accumulate):
  phase 1: qkT = wqkT.T @ xT   (q/k transposed layouts, head pairs stacked
           on partitions), v = xT.T @ wvT (natural layout, with a ones
           column appended per head for fused softmax-denominator)
  phase 2: per head pair / sq-block: scoresT = kT.T @ qT -> exp (ACT,
           fused 1/sqrt(hd) scale) -> valuesT(+denom) = v_ext.T @ attnT
  phase 3: divide by denom (reciprocal + PE ones-broadcast + DVE mul)
  phase 4: o_part = valuesT.T @ owT -> DMA out
"""

import numpy as np

import concourse.bass as bass  # noqa: F401
import concourse.mybir as mybir
import concourse.tile as tile
from concourse import bacc
from concourse.bass_utils import run_bass_kernel_spmd

f32 = mybir.dt.float32
f32r = mybir.dt.float32r
bf16 = mybir.dt.bfloat16
AF = mybir.ActivationFunctionType

import os as _os
MM_DT = f32r if _os.environ.get("MM_DT", "bf16") == "f32r" else bf16

P = 128
N, S, D, H = 2, 2048, 1024, 16
HD = D // H                    # 64
NH = 4                         # heads per core
SCALE = float(1.0 / np.sqrt(np.float32(HD)))
E_QK = 2 * NH * HD             # 512 qk rows per core
E_V = NH * HD                  # 256
DL = NH * HD                   # 256 local d for o_proj
SQB = 512                      # sq block
NSQB = S // SQB                # 4
SKT = S // P                   # 16 sk tiles

# tuning knobs
SKG = int(_os.environ.get("SKG", "1"))        # sk tiles per exp group
SC_BUFS = int(_os.environ.get("SC_BUFS", "2"))
VP_BUFS = int(_os.environ.get("VP_BUFS", "3"))
MP_BUFS = int(_os.environ.get("MP_BUFS", "1"))
ATTN_BUFS = int(_os.environ.get("ATTN_BUFS", "4"))
LAG = int(_os.environ.get("LAG", "2"))   # values matmul lag behind exp


def _emit_body(nc, tc, t, rep):
    from contextlib import ExitStack

    with ExitStack() as ctx:
        const = ctx.enter_context(tc.tile_pool(name=f"const{rep}", bufs=1))
        persist = ctx.enter_context(tc.tile_pool(name=f"persist{rep}", bufs=1))

        wqk_sb = const.tile([P, 8, E_QK], MM_DT, name="wqk_sb")
        wqk_r = t["wqkT"].rearrange("(a p) e -> p a e", p=P)
        for a in range(8):
            nc.scalar.dma_start(wqk_sb[:, a, :], wqk_r[:, a, :])
        wv_sb = const.tile([P, 8, E_V], MM_DT, name="wv_sb")
        nc.gpsimd.dma_start(wv_sb[:], t["wvT"].rearrange("(a p) e -> p a e", p=P))
        ow_sb = const.tile([P, 2, D], MM_DT, name="ow_sb")
        ones_sb = const.tile([65, HD], MM_DT, name="ones_sb")

        qT = persist.tile([P, 2, S], MM_DT, name="qT")
        kT = persist.tile([P, 2, S], MM_DT, name="kT")
        v_sb = persist.tile([P, SKT, NH * 65], MM_DT, name="v_sb")
        vals = persist.tile([P, 2, S], MM_DT, name="vals")
        nc.sync.dma_start(
            v_sb.rearrange("p a (h e) -> p a h e", e=65)[:, :, :, 64:65],
            t["onesd"].rearrange("p (a h) -> p a h", h=NH)[:, :, :, None],
        )

        # ---------------- phase 1: projections ----------------
        with (
            tc.tile_pool(name=f"xp{rep}", bufs=2) as xp,
            tc.tile_pool(name=f"ps1{rep}", bufs=3, space="PSUM") as ps1,
        ):
            xT_r = t["xT"].rearrange("(a p) s -> p a s", p=P)
            for sb in range(4):
                ss = slice(sb * 512, (sb + 1) * 512)
                xt = xp.tile([P, 8, 512], MM_DT, name="xt")
                for a in range(8):
                    eng = nc.gpsimd if a % 2 == 0 else nc.scalar
                    eng.dma_start(xt[:, a, :], xT_r[:, a, ss])
                for et in range(4):
                    pq = ps1.tile([P, 512], f32, name="pq", tag="pq")
                    for a in range(8):
                        nc.tensor.matmul(
                            pq[:],
                            wqk_sb[:, a, et * 128:(et + 1) * 128],
                            xt[:, a, :],
                            start=(a == 0),
                            stop=(a == 7),
                        )
                    dst = (qT if et % 2 == 0 else kT)[:, et // 2, ss]
                    nc.any.tensor_copy(dst, pq[:])
                for st in range(4):
                    pv = ps1.tile([P, E_V], f32, name="pv", tag="pv")
                    for a in range(8):
                        nc.tensor.matmul(
                            pv[:],
                            xt[:, a, st * 128:(st + 1) * 128],
                            wv_sb[:, a, :],
                            start=(a == 0),
                            stop=(a == 7),
                        )
                    so = sb * 4 + st
                    v_r = v_sb[:, so].rearrange("p (h e) -> p h e", e=65)
                    pv_r = pv.rearrange("p (h e) -> p h e", e=64)
                    nc.any.tensor_copy(v_r[:, :, 0:64], pv_r)
                    nc.vector.tensor_scalar(
                        out=v_r[:, :, 64:65],
                        in0=pv_r[:, :, 0:1],
                        scalar1=0.0,
                        scalar2=1.0,
                        op0=mybir.AluOpType.mult,
                        op1=mybir.AluOpType.add,
                    )

        # ---------------- phase 2-4: attention + o_proj ----------------
        nc.sync.dma_start(ow_sb[:], t["owT"].rearrange("(a p) e -> p a e", p=P))
        nc.sync.dma_start(ones_sb[64:65, :], t["onesd"][64:65, 0:HD])
        with (
            tc.tile_pool(name=f"scp{rep}", bufs=SC_BUFS, space="PSUM") as scp,
            tc.tile_pool(name=f"vp{rep}", bufs=VP_BUFS, space="PSUM") as vp,
            tc.tile_pool(name=f"mp{rep}", bufs=MP_BUFS, space="PSUM") as mp,
            tc.tile_pool(name=f"attn{rep}", bufs=ATTN_BUFS) as attnp,
            tc.tile_pool(name=f"sm{rep}", bufs=2) as sm,
            tc.tile_pool(name=f"outp{rep}", bufs=3) as outp,
        ):
            for qb in range(NSQB):
                sqs = slice(qb * SQB, (qb + 1) * SQB)
                for pr in range(2):
                    vps = [
                        vp.tile([65, SQB], f32, name=f"vps{h}", tag="vps")
                        for h in range(2)
                    ]
                    at_tiles = {}
                    ngroups = SKT // SKG
                    for g in range(ngroups + LAG):
                        if g < ngroups:
                            sc = scp.tile([P, SKG, 2, SQB], f32, name="sc",
                                          tag="sc")
                            at = attnp.tile([P, SKG, 2, SQB], MM_DT, name="at",
                                            tag="at")
                            for j in range(SKG):
                                sk = g * SKG + j
                                for h in range(2):
                                    nc.tensor.matmul(
                                        sc[:, j, h, :],
                                        kT[h * 64:(h + 1) * 64, pr,
                                           sk * 128:(sk + 1) * 128],
                                        qT[h * 64:(h + 1) * 64, pr, sqs],
                                        start=True,
                                        stop=True,
                                    )
                            nc.scalar.activation(at[:], sc[:], AF.Exp,
                                                 scale=SCALE)
                            at_tiles[g] = at
                        if g >= LAG:
                            gg = g - LAG
                            atv = at_tiles.pop(gg)
                            for j in range(SKG):
                                sk = gg * SKG + j
                                for h in range(2):
                                    lh = pr * 2 + h
                                    nc.tensor.matmul(
                                        vps[h][:],
                                        v_sb[:, sk, lh * 65:(lh + 1) * 65],
                                        atv[:, j, h, :],
                                        start=(sk == 0),
                                        stop=(sk == SKT - 1),
                                    )
                    for h in range(2):
                        rec_f = sm.tile([65, SQB], f32, name="rec_f", tag="rec_f")
                        nc.vector.reciprocal_approx_fast(
                            rec_f[0:65, :], vps[h][0:65, :]
                        )
                        recr = sm.tile([65, SQB], MM_DT, name="recr", tag="recr")
                        nc.vector.tensor_copy(recr[64:65, :], rec_f[64:65, :])
                        bc = mp.tile([P, SQB], f32, name="bc", tag="m")[0:64, :]
                        nc.tensor.matmul(
                            bc[:],
                            ones_sb[64:65, :],
                            recr[64:65, :],
                            start=True,
                            stop=True,
                        )
                        bcs = sm.tile([64, SQB], f32, name="bcs", tag="bcs")
                        nc.vector.tensor_copy(bcs[:], bc[:])
                        if h == 0:
                            nc.vector.tensor_mul(
                                out=vals[0:64, pr, sqs],
                                in0=vps[h][0:64, :],
                                in1=bcs[:],
                            )
                        else:
                            tmp = sm.tile([64, SQB], MM_DT, name="tmpv", tag="tmpv")
                            nc.vector.tensor_mul(
                                out=tmp[:], in0=vps[h][0:64, :], in1=bcs[:]
                            )
                            nc.sync.dma_start(vals[64:128, pr, sqs], tmp[:])
                # o_proj for the 4 s-tiles of this q block
                for st in range(4):
                    s0 = qb * 4 + st
                    for eb in range(2):
                        ops = mp.tile([P, 512], f32, name="ops", tag="m")
                        for a in range(2):
                            nc.tensor.matmul(
                                ops[:],
                                vals[:, a, s0 * 128:(s0 + 1) * 128],
                                ow_sb[:, a, eb * 512:(eb + 1) * 512],
                                start=(a == 0),
                                stop=(a == 1),
                            )
                        ot = outp.tile([P, 512], f32, name="ot")
                        nc.vector.tensor_copy(ot[:], ops[:])
                        nc.sync.dma_start(
                            t["o"][s0 * 128:(s0 + 1) * 128,
                                   eb * 512:(eb + 1) * 512],
                            ot[:],
                        )


def build_nc(repeats: int = 1):
    nc = bacc.Bacc(None, target_bir_lowering=False)
    t = {
        "xT": nc.dram_tensor("xT", [D, S], MM_DT, kind="ExternalInput")[:, :],
        "wqkT": nc.dram_tensor("wqkT", [D, E_QK], MM_DT, kind="ExternalInput")[:, :],
        "wvT": nc.dram_tensor("wvT", [D, E_V], MM_DT, kind="ExternalInput")[:, :],
        "owT": nc.dram_tensor("owT", [DL, D], MM_DT, kind="ExternalInput")[:, :],
        "onesd": nc.dram_tensor("onesd", [P, SKT * NH], MM_DT,
                                kind="ExternalInput")[:, :],
        "o": nc.dram_tensor("o", [S, D], f32, kind="ExternalOutput")[:, :],
    }
    with tile.TileContext(nc) as tc:
        for rep in range(repeats):
            _emit_body(nc, tc, t, rep)
    nc.compile()
    return nc


def tf32_round(a):
    if MM_DT == bf16:
        import ml_dtypes
        return np.ascontiguousarray(a, dtype=np.float32).astype(ml_dtypes.bfloat16)
    u = np.ascontiguousarray(a, dtype=np.float32).view(np.uint32)
    r = (u + np.uint32(0xFFF) + ((u >> np.uint32(13)) & np.uint32(1))) & ~np.uint32(
        0x1FFF
    )
    return r.view(np.float32)


def make_in_maps(x, qkv_w, o_w):
    x = np.ascontiguousarray(np.asarray(x, dtype=np.float32))
    qkv_w = np.ascontiguousarray(np.asarray(qkv_w, dtype=np.float32))
    o_w = np.ascontiguousarray(np.asarray(o_w, dtype=np.float32))
    in_maps = []
    for c in range(8):
        b, g = c // 4, c % 4
        heads = [4 * g + i for i in range(NH)]
        xT = np.ascontiguousarray(x[b].T)
        wq = [qkv_w[h * 192:h * 192 + 64] for h in heads]
        wk = [qkv_w[h * 192 + 64:h * 192 + 128] for h in heads]
        wv = [qkv_w[h * 192 + 128:h * 192 + 192] for h in heads]
        wqk = np.concatenate(
            [wq[0], wq[1], wk[0], wk[1], wq[2], wq[3], wk[2], wk[3]], axis=0
        )
        wqkT = np.ascontiguousarray(wqk.T)
        wvT = np.ascontiguousarray(np.concatenate(wv, axis=0).T)
        cols = np.concatenate([np.arange(h * 64, h * 64 + 64) for h in heads])
        owT = np.ascontiguousarray(o_w[:, cols].T)
        in_maps.append({"xT": tf32_round(xT), "wqkT": tf32_round(wqkT),
                        "wvT": tf32_round(wvT), "owT": tf32_round(owT),
                        "onesd": tf32_round(np.ones((P, SKT * NH), np.float32))})
    return in_maps


_NC_CACHE = {}


def _get_nc(repeats=1):
    if repeats not in _NC_CACHE:
        _NC_CACHE[repeats] = build_nc(repeats)
    return _NC_CACHE[repeats]


def run_on_hw(x, qkv_w, o_w, repeats=1, **kwargs):
    nc = _get_nc(repeats)
    in_maps = make_in_maps(x, qkv_w, o_w)
    res = run_bass_kernel_spmd(nc, in_maps, core_ids=list(range(8)), **kwargs)
    out = np.zeros((N, S, D), dtype=np.float32)
    for c in range(8):
        out[c // 4] += res.results[c]["o"]
    return out, res


def kernel(x, qkv_w, o_w):
    out, _ = run_on_hw(x, qkv_w, o_w)
    return out

